# revision 1
# baseline (speedup 1.0000x reference)
"""ABMIL (attention-based MIL) Trainium2 kernel, 8-core data-parallel.

Shapes (hardcoded): B=8 bags, N=1024 instances, D=1024, H=16 heads, hd=64.
Each NeuronCore processes one bag. Parameters are replicated (weights are
pre-transposed / bf16-cast on the host where the device layout wants it).

Math: only the cls-token query row survives downstream, so attention is
rank-1 per head and the K/V projections never need materializing:
  znorm      = (z - mu) * rsqrt(var + eps)          rows of z = [cls; x_b]
  ln0        = znorm0 * gamma + beta
  q          = w_q @ ln0 + b_q
  U[h]       = w_k[64h:64h+64].T @ q[64h:64h+64]            (16 x 1024)
  Ug         = U * gamma * 0.125 ;  e_h = 0.125*(U[h]@beta + q_h@b_k_h)
  scores     = Ug @ znorm.T + e                              (16 x 1025)
  A          = softmax(scores)
  Mrow       = (A @ znorm) * gamma + beta                    (16 x 1024)
  ctx[64h:+64] = w_v[64h:+64] @ Mrow[h] + b_v[64h:+64]
  out        = w_o @ ctx + b_o
"""

import numpy as np
import ml_dtypes

import concourse.bass as bass
import concourse.bacc as bacc
import concourse.mybir as mybir
import concourse.tile as tile
from concourse.bass_utils import run_bass_kernel_spmd

F32 = mybir.dt.float32
BF16 = mybir.dt.bfloat16
AX = mybir.AxisListType.X
OP = mybir.AluOpType
AF = mybir.ActivationFunctionType

D = 1024
NK = 8          # number of 128-chunks of D (and of x rows)
H = 16
EPS = 1e-5

_CACHE = {}


def _build():
    nc = bacc.Bacc()

    x_e = nc.declare_dram_parameter("x", [1024, D], BF16, isOutput=False)
    cls_e = nc.declare_dram_parameter("cls", [D], F32, isOutput=False)
    gam_e = nc.declare_dram_parameter("gam", [D], F32, isOutput=False)
    bet_e = nc.declare_dram_parameter("bet", [D], F32, isOutput=False)
    wqT_e = nc.declare_dram_parameter("wqT", [D, D], BF16, isOutput=False)
    wk_e = nc.declare_dram_parameter("wk", [D, D], BF16, isOutput=False)
    wvT_e = nc.declare_dram_parameter("wvT", [D, D], BF16, isOutput=False)
    woT_e = nc.declare_dram_parameter("woT", [D, D], BF16, isOutput=False)
    bq_e = nc.declare_dram_parameter("bq", [D], F32, isOutput=False)
    bk_e = nc.declare_dram_parameter("bk", [D], BF16, isOutput=False)
    bv_e = nc.declare_dram_parameter("bv", [D], F32, isOutput=False)
    bo_e = nc.declare_dram_parameter("bo", [D], BF16, isOutput=False)
    out_e = nc.declare_dram_parameter("out", [1, D], F32, isOutput=True)

    with tile.TileContext(nc) as tc:
        with (
            tc.tile_pool(name="singles", bufs=1) as singles,
            tc.tile_pool(name="xin", bufs=8) as xin,
            tc.tile_pool(name="work", bufs=4) as work,
            tc.tile_pool(name="pt", bufs=2, space="PSUM") as pt,
            tc.tile_pool(name="pbig", bufs=2, space="PSUM") as pbig,
            tc.tile_pool(name="psm", bufs=2, space="PSUM") as psm,
        ):
            # ---- small loads (sync) + broadcasts (gpsimd) --------------
            ident = singles.tile([128, 128], BF16, tag="ident")
            ident_dram = nc.inline_tensor(
                np.eye(128, dtype=ml_dtypes.bfloat16), name="ident_const"
            )
            nc.sync.dma_start(out=ident[:, :], in_=ident_dram[:, :])

            eps_t = singles.tile([128, 1], F32, tag="eps")
            nc.vector.memset(eps_t[:, :], EPS)

            cls_row = singles.tile([1, D], F32, tag="clsr")
            nc.sync.dma_start(out=cls_row[:, :], in_=cls_e[None, :])
            gam_col = singles.tile([128, NK], F32, tag="gamc")
            nc.sync.dma_start(out=gam_col[:, :], in_=gam_e[:].rearrange("(c p) -> p c", p=128))
            bet_col = singles.tile([128, NK], F32, tag="betc")
            nc.sync.dma_start(out=bet_col[:, :], in_=bet_e[:].rearrange("(c p) -> p c", p=128))
            bq_col = singles.tile([128, NK], F32, tag="bqc")
            nc.sync.dma_start(out=bq_col[:, :], in_=bq_e[:].rearrange("(c p) -> p c", p=128))

            # big persistent tiles
            wq_all = singles.tile([128, NK * D], BF16, tag="wq")
            wk_all = singles.tile([128, NK * D], BF16, tag="wk")
            wv_all = singles.tile([128, NK * D], BF16, tag="wv")
            wo_all = singles.tile([128, NK * D], BF16, tag="wo")
            znorm_all = singles.tile([128, NK * D], BF16, tag="znorm")
            znT_all = singles.tile([128, NK * D], BF16, tag="znT")

            def wload(dst_all, src_e, half):
                # one DMA per 512 dram rows: dst[p, c, :] = src[128c+p, :]
                d3 = dst_all[:, :].rearrange("p (c i) -> p c i", c=NK)
                nc.sync.dma_start(
                    out=d3[:, 4 * half : 4 * (half + 1), :],
                    in_=src_e[512 * half : 512 * (half + 1), :].rearrange(
                        "(c p) i -> p c i", p=128
                    ),
                )

            # x chunk DMAs + weight DMAs interleaved on sync queue
            xks = [xin.tile([128, D], BF16, tag="xk", name=f"xk{i}") for i in range(NK)]
            nc.sync.dma_start(out=xks[0][:, :], in_=x_e[0:128, :])
            nc.sync.dma_start(out=xks[1][:, :], in_=x_e[128:256, :])
            wload(wq_all, wqT_e, 0)
            nc.sync.dma_start(out=xks[2][:, :], in_=x_e[256:384, :])
            wload(wq_all, wqT_e, 1)
            nc.sync.dma_start(out=xks[3][:, :], in_=x_e[384:512, :])
            nc.sync.dma_start(out=xks[4][:, :], in_=x_e[512:640, :])
            wload(wk_all, wk_e, 0)
            nc.sync.dma_start(out=xks[5][:, :], in_=x_e[640:768, :])
            wload(wk_all, wk_e, 1)
            nc.sync.dma_start(out=xks[6][:, :], in_=x_e[768:896, :])
            nc.sync.dma_start(out=xks[7][:, :], in_=x_e[896:1024, :])
            bk_col = singles.tile([128, NK], BF16, tag="bkc")
            nc.sync.dma_start(out=bk_col[:, :], in_=bk_e[:].rearrange("(c p) -> p c", p=128))
            wload(wv_all, wvT_e, 0)
            wload(wv_all, wvT_e, 1)
            bv_col = singles.tile([128, NK], F32, tag="bvc")
            nc.sync.dma_start(out=bv_col[:, :], in_=bv_e[:].rearrange("(c p) -> p c", p=128))
            wload(wo_all, woT_e, 0)
            wload(wo_all, woT_e, 1)
            bo_row = singles.tile([1, D], BF16, tag="bor")
            nc.sync.dma_start(out=bo_row[:, :], in_=bo_e[None, :])

            gam16 = singles.tile([H, D], F32, tag="gam16")
            nc.gpsimd.dma_start(
                out=gam16[:, :],
                in_=bass.AP(tensor=gam_e[:].tensor, offset=0, ap=[[0, H], [1, D]]),
            )
            bet16 = singles.tile([H, D], F32, tag="bet16")
            nc.gpsimd.dma_start(
                out=bet16[:, :],
                in_=bass.AP(tensor=bet_e[:].tensor, offset=0, ap=[[0, H], [1, D]]),
            )

            # ---- cls row LN (first DVE work: critical chain) -----------
            stats0 = work.tile([1, 2, 6], F32, tag="stats0")
            nc.vector.bn_stats(out=stats0[:, 0, :], in_=cls_row[:, 0:512])
            nc.vector.bn_stats(out=stats0[:, 1, :], in_=cls_row[:, 512:1024])
            mv0 = work.tile([1, 2], F32, tag="mv0")
            nc.vector.bn_aggr(out=mv0[:, :], in_=stats0[:, :, :])
            nc.scalar.activation(
                out=mv0[:, 1:2], in_=mv0[:, 1:2], func=AF.Sqrt,
                bias=eps_t[0:1, :], scale=1.0,
            )
            rs0 = work.tile([1, 1], F32, tag="rs0")
            nc.vector.tensor_copy(out=rs0[:, :], in_=mv0[:, 1:2])
            nc.vector.reciprocal(out=rs0[:, :], in_=rs0[:, :])
            zn0_row = singles.tile([1, D], BF16, tag="zn0r")
            nc.vector.tensor_scalar(
                out=zn0_row[:, :], in0=cls_row[:, :],
                scalar1=mv0[:, 0:1], scalar2=rs0[:, 0:1],
                op0=OP.subtract, op1=OP.mult,
            )

            # ---- znorm0 column layout + q (gamma/beta applied in cols) --
            lzp = pt.tile([128, 16], BF16, tag="pt")
            for c in range(NK):
                nc.tensor.transpose(
                    out=lzp[:, 2 * c : 2 * c + 1],
                    in_=zn0_row[0:1, 128 * c : 128 * (c + 1)],
                    identity=ident[0:1, 0:1],
                )
            zn0_col = singles.tile([128, NK], BF16, tag="zn0c")
            nc.scalar.copy(
                out=zn0_col[:, :],
                in_=lzp[:, :].rearrange("p (c x) -> p c x", c=NK)[:, :, 0],
            )
            ln0_col = singles.tile([128, NK], BF16, tag="ln0c")
            nc.vector.tensor_mul(out=ln0_col[:, :], in0=zn0_col[:, :], in1=gam_col[:, :])
            nc.vector.tensor_add(out=ln0_col[:, :], in0=ln0_col[:, :], in1=bet_col[:, :])

            psq = pbig.tile([1, D], F32, tag="pbig")
            for c in range(NK):
                for half in range(2):
                    nc.tensor.matmul(
                        psq[:, 512 * half : 512 * (half + 1)], lhsT=ln0_col[:, c : c + 1],
                        rhs=wq_all[:, D * c + 512 * half : D * c + 512 * (half + 1)],
                        start=(c == 0), stop=(c == NK - 1),
                        skip_group_check=True,
                    )
            q_sb = singles.tile([1, D], BF16, tag="qsb")
            nc.scalar.copy(out=q_sb[:, :], in_=psq[:, :])

            qcp = pt.tile([128, 16], BF16, tag="pt")
            for c in range(NK):
                nc.tensor.transpose(
                    out=qcp[:, 2 * c : 2 * c + 1],
                    in_=q_sb[0:1, 128 * c : 128 * (c + 1)],
                    identity=ident[0:1, 0:1],
                )
            q_col = singles.tile([128, NK], BF16, tag="qcol")
            nc.scalar.copy(
                out=q_col[:, :],
                in_=qcp[:, :].rearrange("p (c x) -> p c x", c=NK)[:, :, 0],
            )
            nc.vector.tensor_add(out=q_col[:, :], in0=q_col[:, :], in1=bq_col[:, :])
            qbT = singles.tile([128, H * NK], BF16, tag="qbT")
            nc.gpsimd.memset(qbT[:, :], 0.0)
            for c in range(NK):
                nc.gpsimd.tensor_copy(
                    out=qbT[0:64, H * c + 2 * c : H * c + 2 * c + 1],
                    in_=q_col[0:64, c : c + 1],
                )
                nc.gpsimd.tensor_copy(
                    out=qbT[64:128, H * c + 2 * c + 1 : H * c + 2 * c + 2],
                    in_=q_col[64:128, c : c + 1],
                )

            # ---- U = Qblk @ w_k ; Ug, e --------------------------------
            psU = pbig.tile([H, D], F32, tag="pbig")
            for c in range(NK):
                for half in range(2):
                    nc.tensor.matmul(
                        psU[:, 512 * half : 512 * (half + 1)], lhsT=qbT[:, H * c : H * (c + 1)],
                        rhs=wk_all[:, D * c + 512 * half : D * c + 512 * (half + 1)],
                        start=(c == 0), stop=(c == NK - 1),
                        skip_group_check=True,
                    )
            ug = singles.tile([H, D], BF16, tag="ug")
            nc.vector.scalar_tensor_tensor(
                out=ug[:, :], in0=psU[:, :], scalar=0.125, in1=gam16[:, :],
                op0=OP.mult, op1=OP.mult,
            )
            tmp16 = work.tile([H, D], F32, tag="tmp16")
            nc.vector.tensor_mul(out=tmp16[:, :], in0=psU[:, :], in1=bet16[:, :])
            e1 = work.tile([H, 1], F32, tag="e1")
            nc.vector.reduce_sum(out=e1[:, :], in_=tmp16[:, :], axis=AX)
            pse2 = psm.tile([H, 1], F32, tag="psm")
            for c in range(NK):
                nc.tensor.matmul(
                    pse2[:, :], lhsT=qbT[:, H * c : H * (c + 1)], rhs=bk_col[:, c : c + 1],
                    start=(c == 0), stop=(c == NK - 1),
                )
            e_sb = singles.tile([H, 1], F32, tag="esb")
            nc.vector.tensor_add(out=e_sb[:, :], in0=e1[:, :], in1=pse2[:, :])
            nc.vector.tensor_scalar_mul(out=e_sb[:, :], in0=e_sb[:, :], scalar1=0.125)

            ugp = pt.tile([128, 128], BF16, tag="pt")
            for c in range(NK):
                nc.tensor.transpose(
                    out=ugp[:, H * c : H * (c + 1)], in_=ug[:, 128 * c : 128 * (c + 1)],
                    identity=ident[0:H, 0:H],
                )
            ugT = singles.tile([128, H * NK], BF16, tag="ugT")
            nc.scalar.copy(out=ugT[:, :], in_=ugp[:, :])

            # per-head safe softmax shift: bound_h = 8*||Ug_h|| >= max score
            # (znorm rows have L2 norm sqrt(D)=32; statistically max ~ 3.7*sigma,
            #  sigma ~ ||Ug_h||, so 8*sigma is a safe, non-underflowing bound)
            u2 = work.tile([H, D], F32, tag="u2")
            nc.vector.tensor_mul(out=u2[:, :], in0=ug[:, :], in1=ug[:, :])
            s2 = work.tile([H, 1], F32, tag="s2")
            nc.vector.reduce_sum(out=s2[:, :], in_=u2[:, :], axis=AX)
            bound = work.tile([H, 1], F32, tag="bound")
            nc.scalar.activation(
                out=bound[:, :], in_=s2[:, :], func=AF.Sqrt, bias=0.0, scale=64.0
            )
            eb = work.tile([H, 1], F32, tag="eb")
            nc.vector.tensor_sub(out=eb[:, :], in0=e_sb[:, :], in1=bound[:, :])

            # ---- cls score / attention prologue ------------------------
            a_sb = singles.tile([H, 1025], BF16, tag="asb")
            aT = singles.tile([128, H * NK], BF16, tag="aT")
            se_all = work.tile([H, NK], F32, tag="seall")
            nc.vector.memset(se_all[:, :], 0.0)
            se0 = work.tile([H, 1], F32, tag="se0")
            psS0 = psm.tile([H, 1], F32, tag="psm")
            for c in range(NK):
                nc.tensor.matmul(
                    psS0[:, :], lhsT=ugT[:, H * c : H * (c + 1)], rhs=zn0_col[:, c : c + 1],
                    start=(c == 0), stop=(c == NK - 1),
                )
            nc.scalar.activation(
                out=a_sb[:, 0:1], in_=psS0[:, :], func=AF.Exp,
                bias=eb[:, 0:1], scale=1.0, accum_out=se0[:, :],
            )
            a0p = pt.tile([128, 16], BF16, tag="pt")
            nc.tensor.transpose(out=a0p[0:1, 0:H], in_=a_sb[:, 0:1], identity=ident[0:H, 0:H])
            aT0 = singles.tile([1, H], BF16, tag="aT0")
            nc.scalar.copy(out=aT0[:, :], in_=a0p[0:1, 0:H])
            psM = pbig.tile([H, D], F32, tag="pbig")
            for half in range(2):
                nc.tensor.matmul(
                    psM[:, 512 * half : 512 * (half + 1)], lhsT=aT0[:, :],
                    rhs=zn0_row[:, 512 * half : 512 * (half + 1)],
                    start=True, stop=False, skip_group_check=True,
                )

            # ---- x LayerNorm + streamed scores/softmax/M per chunk -----
            # (static per-head softmax shift => no running max, no rescale)
            for k in range(NK):
                xk = xks[k]
                stats = work.tile([128, 2, 6], F32, tag="stats")
                nc.vector.bn_stats(out=stats[:, 0, :], in_=xk[:, 0:512])
                nc.vector.bn_stats(out=stats[:, 1, :], in_=xk[:, 512:1024])
                mv = work.tile([128, 2], F32, tag="mv")
                nc.vector.bn_aggr(out=mv[:, :], in_=stats[:, :, :])
                nc.scalar.activation(
                    out=mv[:, 1:2], in_=mv[:, 1:2], func=AF.Sqrt,
                    bias=eps_t[:, :], scale=1.0,
                )
                rs = work.tile([128, 1], F32, tag="rs")
                nc.vector.tensor_copy(out=rs[:, :], in_=mv[:, 1:2])
                nc.vector.reciprocal(out=rs[:, :], in_=rs[:, :])
                nc.vector.tensor_scalar(
                    out=znorm_all[:, D * k : D * (k + 1)], in0=xk[:, :],
                    scalar1=mv[:, 0:1], scalar2=rs[:, 0:1],
                    op0=OP.subtract, op1=OP.mult,
                )
                dst3 = znT_all[:, D * k : D * (k + 1)].rearrange(
                    "p (c s) -> p c s", c=NK
                )
                nc.sync.dma_start_transpose(
                    out=dst3, in_=znorm_all[:, D * k : D * (k + 1)]
                )
                # scores for this s-chunk
                psSk = psm.tile([H, 128], F32, tag="psm", name=f"psS{k}")
                for c in range(NK):
                    nc.tensor.matmul(
                        psSk[:, :], lhsT=ugT[:, H * c : H * (c + 1)],
                        rhs=znT_all[:, D * k + 128 * c : D * k + 128 * (c + 1)],
                        start=(c == 0), stop=(c == NK - 1),
                    )
                nc.scalar.activation(
                    out=a_sb[:, 1 + 128 * k : 1 + 128 * (k + 1)], in_=psSk[:, :],
                    func=AF.Exp, bias=eb[:, 0:1], scale=1.0,
                    accum_out=se_all[:, k : k + 1],
                )
                atpk = pt.tile([128, 16], BF16, tag="pt", name=f"atp{k}")
                nc.tensor.transpose(
                    out=atpk[:, 0:H],
                    in_=a_sb[:, 1 + 128 * k : 1 + 128 * (k + 1)],
                    identity=ident[0:H, 0:H],
                )
                nc.scalar.copy(out=aT[:, H * k : H * (k + 1)], in_=atpk[:, 0:H])
                for half in range(2):
                    nc.tensor.matmul(
                        psM[:, 512 * half : 512 * (half + 1)], lhsT=aT[:, H * k : H * (k + 1)],
                        rhs=znorm_all[:, D * k + 512 * half : D * k + 512 * (half + 1)],
                        start=False, stop=(k == NK - 1), skip_group_check=True,
                    )

            rinv = work.tile([H, 1], F32, tag="rinv")
            nc.vector.reduce_sum(out=rinv[:, :], in_=se_all[:, :], axis=AX)
            nc.vector.tensor_add(out=rinv[:, :], in0=rinv[:, :], in1=se0[:, :])
            nc.vector.reciprocal(out=rinv[:, :], in_=rinv[:, :])

            mrow = singles.tile([H, D], BF16, tag="mrow")
            nc.vector.scalar_tensor_tensor(
                out=mrow[:, :], in0=psM[:, :], scalar=rinv[:, 0:1], in1=gam16[:, :],
                op0=OP.mult, op1=OP.mult,
            )
            mtp = pt.tile([128, 128], BF16, tag="pt")
            for c in range(NK):
                nc.tensor.transpose(
                    out=mtp[:, H * c : H * (c + 1)], in_=mrow[:, 128 * c : 128 * (c + 1)],
                    identity=ident[0:H, 0:H],
                )
            mT = singles.tile([128, H * NK], BF16, tag="mT")
            nc.scalar.copy(out=mT[:, :], in_=mtp[:, :])
            for c in range(NK):
                nc.vector.tensor_scalar_add(
                    out=mT[:, H * c : H * (c + 1)], in0=mT[:, H * c : H * (c + 1)],
                    scalar1=bet_col[:, c : c + 1],
                )

            # ---- ctx (fp8 weights, prescaled x16 on host) ---------------
            # ctx column a -> immediately feed the out-projection matmuls, so the
            # out phase hides under the remaining ctx matmuls.
            ctx_sb = singles.tile([128, NK], BF16, tag="ctxsb")
            psO = pbig.tile([1, D], F32, tag="pbig")
            for a in range(NK):
                pc2 = psm.tile([128, 2], F32, tag="psm")
                for c in range(NK):
                    nc.tensor.matmul(
                        pc2[:, :],
                        lhsT=wv_all[:, D * c + 128 * a : D * c + 128 * (a + 1)],
                        rhs=mT[:, H * c + 2 * a : H * c + 2 * a + 2],
                        start=(c == 0), stop=(c == NK - 1),
                    )
                nc.vector.scalar_tensor_tensor(
                    out=ctx_sb[0:64, a : a + 1], in0=pc2[0:64, 0:1], scalar=1.0,
                    in1=bv_col[0:64, a : a + 1], op0=OP.mult, op1=OP.add,
                )
                nc.vector.scalar_tensor_tensor(
                    out=ctx_sb[64:128, a : a + 1], in0=pc2[64:128, 1:2], scalar=1.0,
                    in1=bv_col[64:128, a : a + 1], op0=OP.mult, op1=OP.add,
                )
                for half in range(2):
                    nc.tensor.matmul(
                        psO[:, 512 * half : 512 * (half + 1)],
                        lhsT=ctx_sb[:, a : a + 1],
                        rhs=wo_all[:, D * a + 512 * half : D * a + 512 * (half + 1)],
                        start=(a == 0), stop=False,
                        skip_group_check=True,
                    )
            for half in range(2):
                nc.tensor.matmul(
                    psO[:, 512 * half : 512 * (half + 1)], lhsT=ident[0:1, 0:1],
                    rhs=bo_row[0:1, 512 * half : 512 * (half + 1)],
                    start=False, stop=True, skip_group_check=True,
                )
            out_sb = singles.tile([1, D], F32, tag="outsb")
            nc.scalar.copy(out=out_sb[:, :], in_=psO[:, :])
            nc.sync.dma_start(out=out_e[:, :], in_=out_sb[:, :])

    nc.compile()
    return nc


def _prep_in_maps(inputs):
    bf = ml_dtypes.bfloat16
    f32 = np.float32

    def c(a, dt):
        return np.ascontiguousarray(np.asarray(a), dtype=dt)

    x = c(inputs["x"], bf)
    shared = {
        "cls": c(inputs["cls_token"], f32),
        "gam": c(inputs["gamma"], f32),
        "bet": c(inputs["beta"], f32),
        "wqT": c(np.asarray(inputs["w_q"]).T, bf),
        "wk": c(inputs["w_k"], bf),
        "wvT": c(np.asarray(inputs["w_v"]).T, bf),
        "woT": c(np.asarray(inputs["w_o"]).T, bf),
        "bq": c(inputs["b_q"], f32),
        "bk": c(inputs["b_k"], bf),
        "bv": c(inputs["b_v"], f32),
        "bo": c(inputs["b_o"], bf),
    }
    return [{"x": x[b], **shared} for b in range(8)]


def run(inputs, trace=False, **kw):
    if "nc" not in _CACHE:
        _CACHE["nc"] = _build()
    nc = _CACHE["nc"]
    in_maps = _prep_in_maps(inputs)
    res = run_bass_kernel_spmd(nc, in_maps, core_ids=list(range(8)), trace=trace, **kw)
    out = np.stack([np.asarray(res.results[b]["out"][0], dtype=np.float32) for b in range(8)])
    return out, res


def kernel(**inputs):
    out, _ = run(inputs, trace=False)
    return out

